# revision 59
# baseline (speedup 1.0000x reference)
"""Multi-head causal attention (B=4, S=2048, D=512, H=8) on 8 trn2 NeuronCores.

Sharding: core c -> batch b = c//2, head group hg = c%2 (heads 4*hg .. 4*hg+3).
Each core computes its 4 heads' attention and a partial output projection
(sum over its heads of out_h @ Wo[h-rows]); host sums the two partials per
batch.  The hg==1 core receives a zero bo so the bias is added exactly once.

Device layout (per core; matmul operands bf16, PSUM fp32):
  xT [512, 2048] = x[b].T.  Q^T/K^T per head-pair p are [e2=128, S] with the
  pair's two heads stacked on partition halves; scores are computed
  TRANSPOSED, sT[k, q] = K q^T, with the two heads' K=64-contraction matmuls
  row-tiled into the top/bottom halves of the PE array (concurrent), both
  written into one 2-bank PSUM tile so a single exp (ScalarE, 3D AP) covers
  them.  The causal diagonal 128x128 block is masked by multiplying exp with
  a 0/1 upper-triangular matrix on GpSimd (off the congested DVE queue).
  V' = [V | 1 | pad] is padded to 128 columns (FWL-eligible LDWEIGHTS);
  attn @ V' also yields the softmax normalizer Z in psum row 64.
  Normalization: 1/Z via reciprocal_approx_fast (input staged to a
  base-partition-0 tile first: custom-DVE ops silently misread nonzero base
  partitions on HW), broadcast across partitions by a DRAM round-trip DMA
  (zero-stride read, SWDGE only - keeps the PE out of the softmax path),
  then one tensor_tensor multiply.  Biases bq/bk fold into the PSUM->SBUF
  copy (tensor_scalar), bv/bo are added against partition-broadcast bias
  tiles (DMA'd once).  Emission is software-pipelined: QKV units are
  interleaved into the attention j-loops in deadline order, and each block's
  projection is deferred past the next block's first half so the normalize
  latency hides behind attention work.
"""

import numpy as np
import ml_dtypes

B, S, D, H = 4, 2048, 512, 8
E = D // H  # 64
NCORES = 8
SCALE = float(D) ** -0.5
BF16 = ml_dtypes.bfloat16

_CACHE: dict = {}


def _build_bass():
    import concourse.bass as bass
    import concourse.mybir as mybir
    import concourse.tile as tile
    from concourse import bacc
    from contextlib import ExitStack

    fp32 = mybir.dt.float32
    bf16 = mybir.dt.bfloat16
    Act = mybir.ActivationFunctionType
    Alu = mybir.AluOpType

    nc = bacc.Bacc(
        "TRN2",
        target_bir_lowering=False,
        debug=False,
        num_devices=NCORES,
    )

    xT = nc.dram_tensor("xT", [D, S], bf16, kind="ExternalInput").ap()
    wq = nc.dram_tensor("wq", [2, 4, 128, 128], bf16, kind="ExternalInput").ap()
    wk = nc.dram_tensor("wk", [2, 4, 128, 128], bf16, kind="ExternalInput").ap()
    bq = nc.dram_tensor("bq", [2, 128, 1], fp32, kind="ExternalInput").ap()
    bk = nc.dram_tensor("bk", [2, 128, 1], fp32, kind="ExternalInput").ap()
    wv = nc.dram_tensor("wv", [4, 128, 256], bf16, kind="ExternalInput").ap()
    bvb = nc.dram_tensor("bvb", [1, 256], fp32, kind="ExternalInput").ap()
    wo = nc.dram_tensor("wo", [2, 128, 512], bf16, kind="ExternalInput").ap()
    bob = nc.dram_tensor("bob", [1, 512], fp32, kind="ExternalInput").ap()
    um = nc.dram_tensor("um", [128, 128], bf16, kind="ExternalInput").ap()
    out = nc.dram_tensor("out", [S, D], fp32, kind="ExternalOutput").ap()

    def bcast_ap(src: bass.AP, parts: int, n: int) -> bass.AP:
        """DRAM [1, n] row replicated to [parts, n] via a zero-stride dim."""
        return bass.AP(src.tensor, src.offset, [[0, parts], [1, n]])

    with tile.TileContext(nc) as tc, ExitStack() as ctx:
        const = ctx.enter_context(tc.tile_pool(name="const", bufs=1))
        big = ctx.enter_context(tc.tile_pool(name="big", bufs=1))
        work = ctx.enter_context(tc.tile_pool(name="work", bufs=3))
        psum = ctx.enter_context(tc.tile_pool(name="psum", bufs=2, space="PSUM"))
        dram = ctx.enter_context(tc.tile_pool(name="dram", bufs=2, space="DRAM"))

        # ---- constants / inputs to SBUF (spread across DMA queues) ----
        # Load order matters: the first attention block needs xT s4=0 chunks,
        # wq/wk pair 0, wv, and the bias tiles; spread across the three DMA-
        # capable queues (sync HWDGE, scalar, gpsimd) in first-need order.
        xT_sb = const.tile([128, 4 * S], bf16)  # d-chunk major
        wq_sb = const.tile([128, 8 * 128], bf16)  # (p, dc) major
        wk_sb = const.tile([128, 8 * 128], bf16)
        wv_sb = const.tile([128, 4 * 256], bf16)  # dc major
        wo_sb = const.tile([128, 2 * 512], bf16)  # pair major
        bq_sb = const.tile([128, 2], fp32)
        bk_sb = const.tile([128, 2], fp32)
        um_sb = const.tile([128, 128], bf16)
        bv_bc = const.tile([128, 256], fp32)
        bo_bc = const.tile([128, 512], fp32)
        bv_bc4 = bv_bc.rearrange("d (h e) -> d h e", h=4)

        # one coalesced DMA per tensor (each dma_start costs ~0.6us of queue
        # issue time); nothing on the scalar queue (it must stay free for exp).
        # xT loads are dc-major (contiguous 4KB rows, full DMA bandwidth),
        # split across the sync and gpsimd queues so all four finish early.
        def load_xt(dc, eng):
            eng.dma_start(
                xT_sb[:, dc * S : (dc + 1) * S], xT[dc * 128 : (dc + 1) * 128, :]
            )

        load_xt(0, nc.sync)
        load_xt(1, nc.gpsimd)
        load_xt(2, nc.sync)
        load_xt(3, nc.gpsimd)
        nc.sync.dma_start(
            wq_sb.rearrange("d (a e) -> d a e", a=8),
            wq.rearrange("p c d e -> d (p c) e"),
        )
        nc.sync.dma_start(
            wk_sb.rearrange("d (a e) -> d a e", a=8),
            wk.rearrange("p c d e -> d (p c) e"),
        )
        nc.gpsimd.dma_start(bv_bc, bcast_ap(bvb, 128, 256))
        nc.gpsimd.dma_start(
            wv_sb.rearrange("d (a e) -> d a e", a=4),
            wv.rearrange("c d e -> d c e"),
        )
        nc.gpsimd.dma_start(
            bq_sb, bq.rearrange("p d e -> d (p e)")
        )
        nc.gpsimd.dma_start(
            bk_sb, bk.rearrange("p d e -> d (p e)")
        )
        nc.gpsimd.dma_start(um_sb, um)
        nc.gpsimd.dma_start(
            wo_sb.rearrange("d (a e) -> d a e", a=2),
            wo.rearrange("p d e -> d p e"),
        )
        nc.gpsimd.dma_start(bo_bc, bcast_ap(bob, 128, 512))

        # ---- persistent intermediates ----
        qT_sb = big.tile([128, 2 * S], bf16)  # pair-major; head halves on partitions
        kT_sb = big.tile([128, 2 * S], bf16)
        # V' chunks padded to 128 columns so LDWEIGHTS qualifies for FWL
        # (background weight-buffer load needs a full-128-column stationary
        # operand); col 64 = ones (softmax Z), cols 65..127 = don't-care.
        Vp_sb = big.tile([128, 4 * 16 * 128], bf16)  # (head, k-chunk) major
        Vp4 = Vp_sb.rearrange("d (h j e) -> d h j e", h=4, j=16)
        outT0 = big.tile([128, S], bf16)
        outT1 = big.tile([128, S], bf16)
        outT = [outT0, outT1]

        # ones column (col 64 of each V' chunk) for the softmax normalizer;
        # cols 65..127 are FWL padding whose psum rows are never read.
        nc.vector.memset(Vp4[:, :, :, 64:128], 1.0)
        onef = const.tile([1, 64], fp32)  # fp32 ones row for tail PE-broadcast
        nc.vector.memset(onef, 1.0)

        # ---- QKV ----
        def emit_qk(p, which, s4):
            w_sb, b_sb, scl, dst, tag = (
                (wq_sb, bq_sb, SCALE, qT_sb, "o0")
                if which == "q"
                else (wk_sb, bk_sb, 1.0, kT_sb, "o1")
            )
            mm_ps = psum.tile([128, 512], fp32, tag=tag, name="mm_ps")
            for dc in range(4):
                i = p * 4 + dc
                nc.tensor.matmul(
                    mm_ps,
                    lhsT=w_sb[:, i * 128 : (i + 1) * 128],
                    rhs=xT_sb[:, dc * S + s4 * 512 : dc * S + s4 * 512 + 512],
                    start=(dc == 0),
                    stop=(dc == 3),
                )
            nc.vector.tensor_scalar(
                out=dst[:, p * S + s4 * 512 : p * S + s4 * 512 + 512],
                in0=mm_ps,
                scalar1=scl,
                scalar2=b_sb[:, p : p + 1],
                op0=Alu.mult,
                op1=Alu.add,
            )

        def emit_v(st):
            v_ps = psum.tile(
                [128, 256], fp32, tag=("o0" if st % 2 == 0 else "o1"), name="v_ps"
            )
            for dc in range(4):
                nc.tensor.matmul(
                    v_ps,
                    lhsT=xT_sb[:, dc * S + st * 128 : dc * S + st * 128 + 128],
                    rhs=wv_sb[:, dc * 256 : (dc + 1) * 256],
                    start=(dc == 0),
                    stop=(dc == 3),
                )
            v4 = v_ps.rearrange("d (h e) -> d h e", h=4)
            nc.vector.tensor_tensor(
                out=Vp4[:, :, st, 0:64], in0=v4, in1=bv_bc4, op=Alu.add
            )

        # ---- attention (transposed scores) + deferred projection ----
        def emit_proj(m4):
            for st in range(4 * m4, 4 * m4 + 4):
                pr = psum.tile(
                    [128, 512], fp32, tag=("o0" if st % 2 == 0 else "o1"), name="pr"
                )
                for p in range(2):
                    nc.tensor.matmul(
                        pr,
                        lhsT=outT[p][:, st * 128 : (st + 1) * 128],
                        rhs=wo_sb[:, p * 512 : (p + 1) * 512],
                        start=(p == 0),
                        stop=(p == 1),
                    )
                pr_sb = work.tile([128, 512], fp32, tag="pr_sb", name="pr_sb")
                nc.vector.tensor_tensor(out=pr_sb, in0=pr, in1=bo_bc, op=Alu.add)
                nc.sync.dma_start(out[st * 128 : (st + 1) * 128, :], pr_sb)

        def emit_attn(m4, p, fill=None):
                nj = 4 * (m4 + 1)
                o_ps = [
                    psum.tile([128, 512], fp32, tag=f"o{hi}", name=f"o_ps{hi}")
                    for hi in range(2)
                ]
                def c0_of(j):
                    return 128 * (j - 4 * m4) if j >= 4 * m4 else 0

                for jj in range(0, nj, 2):
                    jpair = (jj, jj + 1)
                    scps = {}
                    exs = {}
                    # both j's, both heads' transposed scores: 4 matmuls in a
                    # row (same kind pipelines; hi pair is row-tile concurrent)
                    for j in jpair:
                        c0 = c0_of(j)
                        scps[j] = psum.tile(
                            [128, 1024], fp32, tag="scp", name="scp"
                        )
                        for hi in range(2):
                            hp = 64 * hi
                            nc.tensor.matmul(
                                scps[j][:, 512 * hi + c0 : 512 * hi + 512],
                                lhsT=kT_sb[
                                    hp : hp + 64,
                                    p * S + j * 128 : p * S + j * 128 + 128,
                                ],
                                rhs=qT_sb[
                                    hp : hp + 64,
                                    p * S + m4 * 512 + c0 : p * S + m4 * 512 + 512,
                                ],
                                start=True,
                                stop=True,
                            )
                    for j in jpair:
                        c0 = c0_of(j)
                        # one exp over both heads' halves (3D AP)
                        ex = work.tile(
                            [128, 1024], bf16, tag="ex", name="ex", bufs=4
                        )
                        exs[j] = ex
                        nc.scalar.activation(
                            ex.rearrange("d (h q) -> d h q", h=2)[:, :, c0:512],
                            scps[j].rearrange("d (h q) -> d h q", h=2)[:, :, c0:512],
                            Act.Exp,
                        )
                        # fill PE slack with pending QKV units during exp
                        for _ in range(2 if m4 == 0 else 1):
                            if fill:
                                fill.popleft()()
                        for hi in range(2):
                            if j >= 4 * m4:
                                # on GpSimd: the DVE queue is congested with
                                # normalize/copy work and this gates attnV
                                nc.gpsimd.tensor_mul(
                                    ex[:, 512 * hi + c0 : 512 * hi + c0 + 128],
                                    ex[:, 512 * hi + c0 : 512 * hi + c0 + 128],
                                    um_sb,
                                )
                    for j in jpair:
                        c0 = c0_of(j)
                        for hi in range(2):
                            nc.tensor.matmul(
                                o_ps[hi][:, c0:512],
                                lhsT=Vp4[:, 2 * p + hi, j, 0:128],
                                rhs=exs[j][:, 512 * hi + c0 : 512 * hi + 512],
                                start=(j == 0),
                                stop=(j == nj - 1),
                                skip_group_check=True,
                            )

                # softmax normalization: Z sits in row 64 of o_ps[hi]; recip it,
                # broadcast across 64 partitions via a DRAM round-trip (no PE),
                # then scale rows 0..63 into the bf16 outT buffer.
                for hi in range(2):
                    # custom-DVE ops require base partition 0: stage Z first
                    zrow = work.tile([1, 512], fp32, tag="zrow", name="zrow")
                    nc.vector.tensor_copy(zrow, o_ps[hi][64:65, :])
                    rc = work.tile([1, 512], fp32, tag="rc", name="rc")
                    nc.vector.reciprocal_approx_fast(rc, zrow)
                    bc = work.tile([64, 512], fp32, tag="bc", name="bc")
                    if m4 == 3 and p == 1:
                        # endgame: PE is idle here, so broadcast 1/Z with an
                        # fp32 ones-matmul instead of the DRAM round-trip
                        # (shorter latency chain into the final projection)
                        bc_ps = psum.tile(
                            [64, 512], fp32, tag=f"o{hi}", name="bc_ps"
                        )
                        nc.tensor.matmul(
                            bc_ps, lhsT=onef, rhs=rc, start=True, stop=True
                        )
                        nc.vector.tensor_copy(bc, bc_ps)
                    else:
                        rcd = dram.tile([1, 512], fp32, tag="rcd", name="rcd")
                        nc.sync.dma_start(rcd, rc)
                        nc.gpsimd.dma_start(bc, bcast_ap(rcd, 64, 512))
                    nc.vector.tensor_mul(
                        outT[p][64 * hi : 64 * hi + 64, m4 * 512 : (m4 + 1) * 512],
                        o_ps[hi][0:64, :],
                        bc,
                    )
        # ---- software-pipelined emission: QKV units are interleaved into the
        # attention j-loops (PE slack) in deadline order; each block's
        # prerequisites are flushed before the block starts. proj of block m4
        # is deferred past (m4+1, p=0) to hide normalize latency.
        from collections import deque
        from functools import partial

        units = deque()
        for st in range(1, 4):
            units.append(partial(emit_v, st))  # V1..V3 (A(0,*) aV j=1..3)
        units.append(partial(emit_qk, 1, "q", 0))
        units.append(partial(emit_qk, 1, "k", 0))
        for m4n in range(1, 4):
            for st in range(4 * m4n, 4 * m4n + 4):
                units.append(partial(emit_v, st))
            units.append(partial(emit_qk, 0, "q", m4n))
            units.append(partial(emit_qk, 0, "k", m4n))
            units.append(partial(emit_qk, 1, "q", m4n))
            units.append(partial(emit_qk, 1, "k", m4n))
        total_units = len(units)
        # units that must be emitted before block (m4, p) starts
        need = {(0, 0): 0, (0, 1): 5}
        for m4 in range(1, 4):
            base = 5 + (m4 - 1) * 8
            need[(m4, 0)] = base + 6
            need[(m4, 1)] = base + 8

        def flush_to(n):
            while total_units - len(units) < n:
                units.popleft()()

        emit_qk(0, "q", 0)
        emit_qk(0, "k", 0)
        emit_v(0)
        pending_proj = None
        for m4 in range(4):
            for p in range(2):
                flush_to(need[(m4, p)])
                emit_attn(m4, p, fill=units)
                if p == 0 and pending_proj is not None:
                    emit_proj(pending_proj)
                    pending_proj = None
            pending_proj = m4
        while units:
            units.popleft()()
        emit_proj(3)

    nc.compile()
    return nc


def _get_bass():
    if "nc" not in _CACHE:
        _CACHE["nc"] = _build_bass()
    return _CACHE["nc"]


def make_in_maps(x, Wq, bq, Wk, bk, Wv, bv, Wo, bo):
    """Pack full fp32 inputs into 8 per-core input dicts."""
    x = np.asarray(x, np.float32)
    Wq = np.asarray(Wq, np.float32)
    bq = np.asarray(bq, np.float32)
    Wk = np.asarray(Wk, np.float32)
    bk = np.asarray(bk, np.float32)
    Wv = np.asarray(Wv, np.float32)
    bv = np.asarray(bv, np.float32)
    Wo = np.asarray(Wo, np.float32)
    bo = np.asarray(bo, np.float32)

    um = np.triu(np.ones((128, 128), np.float32)).astype(BF16)  # keep q >= k
    zeros_bo = np.zeros((1, 512), np.float32)

    in_maps = []
    for c in range(NCORES):
        b = c // 2
        hg = c % 2
        heads = [4 * hg + i for i in range(4)]

        xT_b = np.ascontiguousarray(x[b].T).astype(BF16)  # [512, 2048]

        wq_c = np.empty((2, 4, 128, 128), BF16)
        wk_c = np.empty((2, 4, 128, 128), BF16)
        bq_c = np.empty((2, 128, 1), np.float32)
        bk_c = np.empty((2, 128, 1), np.float32)
        wo_c = np.empty((2, 128, 512), BF16)
        for p in range(2):
            hA, hB = heads[2 * p], heads[2 * p + 1]
            blk_q = np.concatenate([Wq[hA], Wq[hB]], axis=1)  # [512, 128]
            blk_k = np.concatenate([Wk[hA], Wk[hB]], axis=1)
            for dc in range(4):
                wq_c[p, dc] = blk_q[dc * 128 : (dc + 1) * 128].astype(BF16)
                wk_c[p, dc] = blk_k[dc * 128 : (dc + 1) * 128].astype(BF16)
            bq_c[p, :, 0] = np.concatenate([bq[hA], bq[hB]]) * SCALE
            bk_c[p, :, 0] = np.concatenate([bk[hA], bk[hB]])
            wo_c[p] = np.concatenate(
                [Wo[E * hA : E * hA + E], Wo[E * hB : E * hB + E]], axis=0
            ).astype(BF16)

        wv_blk = np.concatenate([Wv[h] for h in heads], axis=1)  # [512, 256]
        wv_c = np.empty((4, 128, 256), BF16)
        for dc in range(4):
            wv_c[dc] = wv_blk[dc * 128 : (dc + 1) * 128].astype(BF16)
        bv_c = np.concatenate([bv[h] for h in heads])[None, :].astype(np.float32)

        in_maps.append(
            {
                "xT": xT_b,
                "wq": wq_c,
                "wk": wk_c,
                "bq": bq_c,
                "bk": bk_c,
                "wv": wv_c,
                "bvb": bv_c,
                "wo": wo_c,
                "bob": bo[None, :].astype(np.float32) if hg == 0 else zeros_bo,
                "um": um,
            }
        )
    return in_maps


def combine_outputs(parts):
    """parts: list of 8 [S, D] fp32 partials -> [B, S, D]."""
    out = np.empty((B, S, D), np.float32)
    for b in range(B):
        np.add(parts[2 * b], parts[2 * b + 1], out=out[b])
    return out


def kernel(**inputs):
    from concourse.bass_utils import run_bass_kernel_spmd

    nc = _get_bass()
    in_maps = make_in_maps(**inputs)
    res = run_bass_kernel_spmd(nc, in_maps, core_ids=list(range(NCORES)))
    return combine_outputs([r["out"] for r in res.results])


# revision 60
# speedup vs baseline: 1.0489x; 1.0489x over previous
"""Multi-head causal attention (B=4, S=2048, D=512, H=8) on 8 trn2 NeuronCores.

Sharding: core c -> batch b = c//2, head group hg = c%2 (heads 4*hg .. 4*hg+3).
Each core computes its 4 heads' attention and a partial output projection
(sum over its heads of out_h @ Wo[h-rows]); host sums the two partials per
batch.  The hg==1 core receives a zero bo so the bias is added exactly once.

Device layout (per core; matmul operands bf16, PSUM fp32):
  xT [512, 2048] = x[b].T.  Q^T/K^T per head-pair p are [e2=128, S] with the
  pair's two heads stacked on partition halves; scores are computed
  TRANSPOSED, sT[k, q] = K q^T, with the two heads' K=64-contraction matmuls
  row-tiled into the top/bottom halves of the PE array (concurrent), both
  written into one 2-bank PSUM tile so a single exp (ScalarE, 3D AP) covers
  them.  The causal diagonal 128x128 block is masked by multiplying exp with
  a 0/1 upper-triangular matrix on GpSimd (off the congested DVE queue).
  V' = [V | 1 | pad] is padded to 128 columns (FWL-eligible LDWEIGHTS);
  attn @ V' also yields the softmax normalizer Z in psum row 64.
  Normalization: 1/Z via reciprocal_approx_fast (input staged to a
  base-partition-0 tile first: custom-DVE ops silently misread nonzero base
  partitions on HW), broadcast across partitions by a DRAM round-trip DMA
  (zero-stride read, SWDGE only - keeps the PE out of the softmax path),
  then one tensor_tensor multiply.  Biases bq/bk fold into the PSUM->SBUF
  copy (tensor_scalar), bv/bo are added against partition-broadcast bias
  tiles (DMA'd once).  Emission is software-pipelined: QKV units are
  interleaved into the attention j-loops in deadline order, and each block's
  projection is deferred past the next block's first half so the normalize
  latency hides behind attention work.
"""

import numpy as np
import ml_dtypes

B, S, D, H = 4, 2048, 512, 8
E = D // H  # 64
NCORES = 8
SCALE = float(D) ** -0.5
BF16 = ml_dtypes.bfloat16

_CACHE: dict = {}


def _build_bass():
    import concourse.bass as bass
    import concourse.mybir as mybir
    import concourse.tile as tile
    from concourse import bacc
    from contextlib import ExitStack

    fp32 = mybir.dt.float32
    bf16 = mybir.dt.bfloat16
    Act = mybir.ActivationFunctionType
    Alu = mybir.AluOpType

    nc = bacc.Bacc(
        "TRN2",
        target_bir_lowering=False,
        debug=False,
        num_devices=NCORES,
    )

    xT = nc.dram_tensor("xT", [D, S], bf16, kind="ExternalInput").ap()
    wq = nc.dram_tensor("wq", [2, 4, 128, 128], bf16, kind="ExternalInput").ap()
    wk = nc.dram_tensor("wk", [2, 4, 128, 128], bf16, kind="ExternalInput").ap()
    bq = nc.dram_tensor("bq", [2, 128, 1], fp32, kind="ExternalInput").ap()
    bk = nc.dram_tensor("bk", [2, 128, 1], fp32, kind="ExternalInput").ap()
    wv = nc.dram_tensor("wv", [4, 128, 256], bf16, kind="ExternalInput").ap()
    bvb = nc.dram_tensor("bvb", [1, 256], fp32, kind="ExternalInput").ap()
    wo = nc.dram_tensor("wo", [2, 128, 512], bf16, kind="ExternalInput").ap()
    bob = nc.dram_tensor("bob", [1, 512], fp32, kind="ExternalInput").ap()
    um = nc.dram_tensor("um", [128, 128], bf16, kind="ExternalInput").ap()
    out = nc.dram_tensor("out", [S, D], fp32, kind="ExternalOutput").ap()

    def bcast_ap(src: bass.AP, parts: int, n: int) -> bass.AP:
        """DRAM [1, n] row replicated to [parts, n] via a zero-stride dim."""
        return bass.AP(src.tensor, src.offset, [[0, parts], [1, n]])

    with tile.TileContext(nc) as tc, ExitStack() as ctx:
        const = ctx.enter_context(tc.tile_pool(name="const", bufs=1))
        big = ctx.enter_context(tc.tile_pool(name="big", bufs=1))
        work = ctx.enter_context(tc.tile_pool(name="work", bufs=3))
        psum = ctx.enter_context(tc.tile_pool(name="psum", bufs=2, space="PSUM"))
        dram = ctx.enter_context(tc.tile_pool(name="dram", bufs=2, space="DRAM"))

        # ---- constants / inputs to SBUF (spread across DMA queues) ----
        # Load order matters: the first attention block needs xT s4=0 chunks,
        # wq/wk pair 0, wv, and the bias tiles; spread across the three DMA-
        # capable queues (sync HWDGE, scalar, gpsimd) in first-need order.
        xT_sb = const.tile([128, 4 * S], bf16)  # d-chunk major
        wq_sb = const.tile([128, 8 * 128], bf16)  # (p, dc) major
        wk_sb = const.tile([128, 8 * 128], bf16)
        wv_sb = const.tile([128, 4 * 256], bf16)  # dc major
        wo_sb = const.tile([128, 2 * 512], bf16)  # pair major
        bq_sb = const.tile([128, 2], fp32)
        bk_sb = const.tile([128, 2], fp32)
        um_sb = const.tile([128, 128], bf16)
        bv_bc = const.tile([128, 256], fp32)
        bo_bc = const.tile([128, 512], fp32)
        bv_bc4 = bv_bc.rearrange("d (h e) -> d h e", h=4)

        # Transfer order is latency-critical at the head: the first attention
        # block needs only the s4=0 column block of xT (all 4 d-chunks), wq/wk
        # pair 0, and wv.  Load exactly those pieces first, split between the
        # sync and gpsimd queues; bulk-load the rest afterwards.  Nothing goes
        # on the scalar queue (it must stay free for exp).
        def load_xt_piece(dc, lo, hi, eng):
            eng.dma_start(
                xT_sb[:, dc * S + lo : dc * S + hi],
                xT[dc * 128 : (dc + 1) * 128, lo:hi],
            )

        load_xt_piece(0, 0, 512, nc.sync)
        load_xt_piece(1, 0, 512, nc.sync)
        load_xt_piece(2, 0, 512, nc.gpsimd)
        load_xt_piece(3, 0, 512, nc.gpsimd)
        nc.sync.dma_start(
            wq_sb[:, 0:512].rearrange("d (a e) -> d a e", a=4),
            wq[0].rearrange("c d e -> d c e"),
        )
        nc.gpsimd.dma_start(
            wk_sb[:, 0:512].rearrange("d (a e) -> d a e", a=4),
            wk[0].rearrange("c d e -> d c e"),
        )
        nc.gpsimd.dma_start(
            wv_sb.rearrange("d (a e) -> d a e", a=4),
            wv.rearrange("c d e -> d c e"),
        )
        nc.gpsimd.dma_start(bv_bc, bcast_ap(bvb, 128, 256))
        for dc in range(4):
            load_xt_piece(dc, 512, 2048, nc.sync)
        nc.sync.dma_start(
            wq_sb[:, 512:1024].rearrange("d (a e) -> d a e", a=4),
            wq[1].rearrange("c d e -> d c e"),
        )
        nc.sync.dma_start(
            wk_sb[:, 512:1024].rearrange("d (a e) -> d a e", a=4),
            wk[1].rearrange("c d e -> d c e"),
        )
        nc.gpsimd.dma_start(
            bq_sb, bq.rearrange("p d e -> d (p e)")
        )
        nc.gpsimd.dma_start(
            bk_sb, bk.rearrange("p d e -> d (p e)")
        )
        nc.gpsimd.dma_start(um_sb, um)
        nc.gpsimd.dma_start(
            wo_sb.rearrange("d (a e) -> d a e", a=2),
            wo.rearrange("p d e -> d p e"),
        )
        nc.gpsimd.dma_start(bo_bc, bcast_ap(bob, 128, 512))

        # ---- persistent intermediates ----
        qT_sb = big.tile([128, 2 * S], bf16)  # pair-major; head halves on partitions
        kT_sb = big.tile([128, 2 * S], bf16)
        # V' chunks padded to 128 columns so LDWEIGHTS qualifies for FWL
        # (background weight-buffer load needs a full-128-column stationary
        # operand); col 64 = ones (softmax Z), cols 65..127 = don't-care.
        Vp_sb = big.tile([128, 4 * 16 * 128], bf16)  # (head, k-chunk) major
        Vp4 = Vp_sb.rearrange("d (h j e) -> d h j e", h=4, j=16)
        outT0 = big.tile([128, S], bf16)
        outT1 = big.tile([128, S], bf16)
        outT = [outT0, outT1]

        # ones column (col 64 of each V' chunk) for the softmax normalizer;
        # cols 65..127 are FWL padding whose psum rows are never read.
        nc.vector.memset(Vp4[:, :, :, 64:128], 1.0)
        onef = const.tile([1, 64], fp32)  # fp32 ones row for tail PE-broadcast
        nc.vector.memset(onef, 1.0)

        # ---- QKV ----
        def emit_qk(p, which, s4):
            w_sb, b_sb, scl, dst, tag = (
                (wq_sb, bq_sb, SCALE, qT_sb, "o0")
                if which == "q"
                else (wk_sb, bk_sb, 1.0, kT_sb, "o1")
            )
            mm_ps = psum.tile([128, 512], fp32, tag=tag, name="mm_ps")
            for dc in range(4):
                i = p * 4 + dc
                nc.tensor.matmul(
                    mm_ps,
                    lhsT=w_sb[:, i * 128 : (i + 1) * 128],
                    rhs=xT_sb[:, dc * S + s4 * 512 : dc * S + s4 * 512 + 512],
                    start=(dc == 0),
                    stop=(dc == 3),
                )
            nc.vector.tensor_scalar(
                out=dst[:, p * S + s4 * 512 : p * S + s4 * 512 + 512],
                in0=mm_ps,
                scalar1=scl,
                scalar2=b_sb[:, p : p + 1],
                op0=Alu.mult,
                op1=Alu.add,
            )

        def emit_v(st):
            v_ps = psum.tile(
                [128, 256], fp32, tag=("o0" if st % 2 == 0 else "o1"), name="v_ps"
            )
            for dc in range(4):
                nc.tensor.matmul(
                    v_ps,
                    lhsT=xT_sb[:, dc * S + st * 128 : dc * S + st * 128 + 128],
                    rhs=wv_sb[:, dc * 256 : (dc + 1) * 256],
                    start=(dc == 0),
                    stop=(dc == 3),
                )
            v4 = v_ps.rearrange("d (h e) -> d h e", h=4)
            nc.vector.tensor_tensor(
                out=Vp4[:, :, st, 0:64], in0=v4, in1=bv_bc4, op=Alu.add
            )

        # ---- attention (transposed scores) + deferred projection ----
        def emit_proj(m4):
            for st in range(4 * m4, 4 * m4 + 4):
                pr = psum.tile(
                    [128, 512], fp32, tag=("o0" if st % 2 == 0 else "o1"), name="pr"
                )
                for p in range(2):
                    nc.tensor.matmul(
                        pr,
                        lhsT=outT[p][:, st * 128 : (st + 1) * 128],
                        rhs=wo_sb[:, p * 512 : (p + 1) * 512],
                        start=(p == 0),
                        stop=(p == 1),
                    )
                pr_sb = work.tile([128, 512], fp32, tag="pr_sb", name="pr_sb")
                nc.vector.tensor_tensor(out=pr_sb, in0=pr, in1=bo_bc, op=Alu.add)
                nc.sync.dma_start(out[st * 128 : (st + 1) * 128, :], pr_sb)

        def emit_attn(m4, p, fill=None):
                nj = 4 * (m4 + 1)
                o_ps = [
                    psum.tile([128, 512], fp32, tag=f"o{hi}", name=f"o_ps{hi}")
                    for hi in range(2)
                ]
                def c0_of(j):
                    return 128 * (j - 4 * m4) if j >= 4 * m4 else 0

                for jj in range(0, nj, 2):
                    jpair = (jj, jj + 1)
                    scps = {}
                    exs = {}
                    # both j's, both heads' transposed scores: 4 matmuls in a
                    # row (same kind pipelines; hi pair is row-tile concurrent)
                    for j in jpair:
                        c0 = c0_of(j)
                        scps[j] = psum.tile(
                            [128, 1024], fp32, tag="scp", name="scp"
                        )
                        for hi in range(2):
                            hp = 64 * hi
                            nc.tensor.matmul(
                                scps[j][:, 512 * hi + c0 : 512 * hi + 512],
                                lhsT=kT_sb[
                                    hp : hp + 64,
                                    p * S + j * 128 : p * S + j * 128 + 128,
                                ],
                                rhs=qT_sb[
                                    hp : hp + 64,
                                    p * S + m4 * 512 + c0 : p * S + m4 * 512 + 512,
                                ],
                                start=True,
                                stop=True,
                            )
                    for j in jpair:
                        c0 = c0_of(j)
                        # one exp over both heads' halves (3D AP)
                        ex = work.tile(
                            [128, 1024], bf16, tag="ex", name="ex", bufs=4
                        )
                        exs[j] = ex
                        nc.scalar.activation(
                            ex.rearrange("d (h q) -> d h q", h=2)[:, :, c0:512],
                            scps[j].rearrange("d (h q) -> d h q", h=2)[:, :, c0:512],
                            Act.Exp,
                        )
                        # fill PE slack with pending QKV units during exp
                        for _ in range(2 if m4 == 0 else 1):
                            if fill:
                                fill.popleft()()
                        for hi in range(2):
                            if j >= 4 * m4:
                                # on GpSimd: the DVE queue is congested with
                                # normalize/copy work and this gates attnV
                                nc.gpsimd.tensor_mul(
                                    ex[:, 512 * hi + c0 : 512 * hi + c0 + 128],
                                    ex[:, 512 * hi + c0 : 512 * hi + c0 + 128],
                                    um_sb,
                                )
                    for j in jpair:
                        c0 = c0_of(j)
                        for hi in range(2):
                            nc.tensor.matmul(
                                o_ps[hi][:, c0:512],
                                lhsT=Vp4[:, 2 * p + hi, j, 0:128],
                                rhs=exs[j][:, 512 * hi + c0 : 512 * hi + 512],
                                start=(j == 0),
                                stop=(j == nj - 1),
                                skip_group_check=True,
                            )

                # softmax normalization: Z sits in row 64 of o_ps[hi]; recip it,
                # broadcast across 64 partitions via a DRAM round-trip (no PE),
                # then scale rows 0..63 into the bf16 outT buffer.
                for hi in range(2):
                    # custom-DVE ops require base partition 0: stage Z first
                    zrow = work.tile([1, 512], fp32, tag="zrow", name="zrow")
                    nc.vector.tensor_copy(zrow, o_ps[hi][64:65, :])
                    rc = work.tile([1, 512], fp32, tag="rc", name="rc")
                    nc.vector.reciprocal_approx_fast(rc, zrow)
                    bc = work.tile([64, 512], fp32, tag="bc", name="bc")
                    if m4 == 3 and p == 1:
                        # endgame: PE is idle here, so broadcast 1/Z with an
                        # fp32 ones-matmul instead of the DRAM round-trip
                        # (shorter latency chain into the final projection)
                        bc_ps = psum.tile(
                            [64, 512], fp32, tag=f"o{hi}", name="bc_ps"
                        )
                        nc.tensor.matmul(
                            bc_ps, lhsT=onef, rhs=rc, start=True, stop=True
                        )
                        nc.vector.tensor_copy(bc, bc_ps)
                    else:
                        rcd = dram.tile([1, 512], fp32, tag="rcd", name="rcd")
                        nc.sync.dma_start(rcd, rc)
                        nc.gpsimd.dma_start(bc, bcast_ap(rcd, 64, 512))
                    nc.vector.tensor_mul(
                        outT[p][64 * hi : 64 * hi + 64, m4 * 512 : (m4 + 1) * 512],
                        o_ps[hi][0:64, :],
                        bc,
                    )
        # ---- software-pipelined emission: QKV units are interleaved into the
        # attention j-loops (PE slack) in deadline order; each block's
        # prerequisites are flushed before the block starts. proj of block m4
        # is deferred past (m4+1, p=0) to hide normalize latency.
        from collections import deque
        from functools import partial

        units = deque()
        for st in range(1, 4):
            units.append(partial(emit_v, st))  # V1..V3 (A(0,*) aV j=1..3)
        units.append(partial(emit_qk, 1, "q", 0))
        units.append(partial(emit_qk, 1, "k", 0))
        for m4n in range(1, 4):
            for st in range(4 * m4n, 4 * m4n + 4):
                units.append(partial(emit_v, st))
            units.append(partial(emit_qk, 0, "q", m4n))
            units.append(partial(emit_qk, 0, "k", m4n))
            units.append(partial(emit_qk, 1, "q", m4n))
            units.append(partial(emit_qk, 1, "k", m4n))
        total_units = len(units)
        # units that must be emitted before block (m4, p) starts
        need = {(0, 0): 0, (0, 1): 5}
        for m4 in range(1, 4):
            base = 5 + (m4 - 1) * 8
            need[(m4, 0)] = base + 6
            need[(m4, 1)] = base + 8

        def flush_to(n):
            while total_units - len(units) < n:
                units.popleft()()

        emit_qk(0, "q", 0)
        emit_qk(0, "k", 0)
        emit_v(0)
        pending_proj = None
        for m4 in range(4):
            for p in range(2):
                flush_to(need[(m4, p)])
                emit_attn(m4, p, fill=units)
                if p == 0 and pending_proj is not None:
                    emit_proj(pending_proj)
                    pending_proj = None
            pending_proj = m4
        while units:
            units.popleft()()
        emit_proj(3)

    nc.compile()
    return nc


def _get_bass():
    if "nc" not in _CACHE:
        _CACHE["nc"] = _build_bass()
    return _CACHE["nc"]


def make_in_maps(x, Wq, bq, Wk, bk, Wv, bv, Wo, bo):
    """Pack full fp32 inputs into 8 per-core input dicts."""
    x = np.asarray(x, np.float32)
    Wq = np.asarray(Wq, np.float32)
    bq = np.asarray(bq, np.float32)
    Wk = np.asarray(Wk, np.float32)
    bk = np.asarray(bk, np.float32)
    Wv = np.asarray(Wv, np.float32)
    bv = np.asarray(bv, np.float32)
    Wo = np.asarray(Wo, np.float32)
    bo = np.asarray(bo, np.float32)

    um = np.triu(np.ones((128, 128), np.float32)).astype(BF16)  # keep q >= k
    zeros_bo = np.zeros((1, 512), np.float32)

    in_maps = []
    for c in range(NCORES):
        b = c // 2
        hg = c % 2
        heads = [4 * hg + i for i in range(4)]

        xT_b = np.ascontiguousarray(x[b].T).astype(BF16)  # [512, 2048]

        wq_c = np.empty((2, 4, 128, 128), BF16)
        wk_c = np.empty((2, 4, 128, 128), BF16)
        bq_c = np.empty((2, 128, 1), np.float32)
        bk_c = np.empty((2, 128, 1), np.float32)
        wo_c = np.empty((2, 128, 512), BF16)
        for p in range(2):
            hA, hB = heads[2 * p], heads[2 * p + 1]
            blk_q = np.concatenate([Wq[hA], Wq[hB]], axis=1)  # [512, 128]
            blk_k = np.concatenate([Wk[hA], Wk[hB]], axis=1)
            for dc in range(4):
                wq_c[p, dc] = blk_q[dc * 128 : (dc + 1) * 128].astype(BF16)
                wk_c[p, dc] = blk_k[dc * 128 : (dc + 1) * 128].astype(BF16)
            bq_c[p, :, 0] = np.concatenate([bq[hA], bq[hB]]) * SCALE
            bk_c[p, :, 0] = np.concatenate([bk[hA], bk[hB]])
            wo_c[p] = np.concatenate(
                [Wo[E * hA : E * hA + E], Wo[E * hB : E * hB + E]], axis=0
            ).astype(BF16)

        wv_blk = np.concatenate([Wv[h] for h in heads], axis=1)  # [512, 256]
        wv_c = np.empty((4, 128, 256), BF16)
        for dc in range(4):
            wv_c[dc] = wv_blk[dc * 128 : (dc + 1) * 128].astype(BF16)
        bv_c = np.concatenate([bv[h] for h in heads])[None, :].astype(np.float32)

        in_maps.append(
            {
                "xT": xT_b,
                "wq": wq_c,
                "wk": wk_c,
                "bq": bq_c,
                "bk": bk_c,
                "wv": wv_c,
                "bvb": bv_c,
                "wo": wo_c,
                "bob": bo[None, :].astype(np.float32) if hg == 0 else zeros_bo,
                "um": um,
            }
        )
    return in_maps


def combine_outputs(parts):
    """parts: list of 8 [S, D] fp32 partials -> [B, S, D]."""
    out = np.empty((B, S, D), np.float32)
    for b in range(B):
        np.add(parts[2 * b], parts[2 * b + 1], out=out[b])
    return out


def kernel(**inputs):
    from concourse.bass_utils import run_bass_kernel_spmd

    nc = _get_bass()
    in_maps = make_in_maps(**inputs)
    res = run_bass_kernel_spmd(nc, in_maps, core_ids=list(range(NCORES)))
    return combine_outputs([r["out"] for r in res.results])
